# revision 17
# baseline (speedup 1.0000x reference)
"""NeuroODE kernel for 8 Trainium2 NeuronCores.

Math: each Euler sub-step is y <- (alpha*I + beta*P) y + gamma*ones, with
P the cyclic shift (roll by 1). Composing the 8 sub-steps of big step n
gives a 9-tap circulant operator W_n; composing across big steps keeps the
state circulant in y0:

    y_n = C_n (*) y0 + s_n * ones

where C_n (tap vector, circular convolution) obeys C_{n+1} = W_n (*) C_n
and the forcing collapses to the scalar recurrence s_{n+1} = lam_n^8 s_n
+ g_n because P*ones = ones (computed on host in f64). The taps are a
binomial bump centered at ~8*n*beta/(alpha+beta), so C_n is supported on
the first TAPS taps, and the full output is the banded product

    Y[n, i] = sum_k C[n, k] * y0[(i - k) mod 2048] + s_n.

The row-normalized tap matrix is a smooth one-parameter family of
binomial bumps with numerical rank ~25, so C = D @ (U S V'); the device
never sees C or the shifted-y0 matrix at all:

    Y = A @ W + s 1',   A = D U S (2048 x R),  W = V' G (R x 2048)

with G[k, i] = y0[(i-k) mod 2048] contracted on the host (tiny, f64).
The bias is folded in as an extra contraction row. Each of the 8 cores
computes 256 output rows (its row block of A against the shared W).

Precision: operands are plain bf16 and A is shipped row-normalized
(A/rn = U S exactly), so the device computes Y/rn and the host scales
rows back by rn after the gather. The shipped output is bf16 for each
core's last 128 rows and fp8-e4m3 for its first 128: the row norms grow
geometrically in n (lam > 1), so the first half of every 256-row block
carries ~3% of that block's output power and tolerates fp8's ~2.7% rms
element error at negligible norm-level cost (measured 5.5e-3 total vs
the f32 reference, against a 2e-2 gate, while cutting the dominant
output-DMA bytes by 25%). If a different parameter draw made the row
norms flat or decaying, the host detects it and falls back to an
all-bf16 variant (measured 2.8e-3).

Schedule (TimelineSim-tuned): operands packed per core as
[lhsT_mc1 | W0 | W1 | lhsT_mc0 | W2 | W3] in one [KP, 2304] bf16
tensor, loaded in two equal DMAs: the first carries everything the
bf16 row block's leading matmuls (lhsT_mc1 x W0-W1) need, so no matmul
stalls on the second DMA's 900ns completion semaphore. The bf16 row block is
computed first (its 2x728ns out-DMAs start the transfer chain early),
the fp8 block second as a 1536+512 split so the last transfer is small
and gated only by the final copy. PSUM->SBUF cast copies alternate
DVE/Act (the only PSUM-capable engines); all out-DMAs go on the SP
queue - the 650ns dispatch + 625ns shared-HWDGE staircase stays just
ahead of the transfers, keeping the DMA engines dense from first
transfer to end.
"""

import math

import numpy as np

SAMPLE_NUM = 2048
Y_NUM = 2048
STEP_N = 8
N_CORES = 8
ROWS_PER_CORE = SAMPLE_NUM // N_CORES  # 256
A0 = ROWS_PER_CORE                     # lhsT columns in the packed input
NM = ROWS_PER_CORE // 128              # 128-row output row blocks

_COMPILED = {}  # (KP, hybrid) -> nc


def _build_bass(KP, hybrid):
    """KP: padded contraction size (rank+bias+pad): 32/64/128."""
    import concourse.tile as tile
    from concourse import bacc, mybir

    f32 = mybir.dt.float32
    bf16 = mybir.dt.bfloat16
    fp8 = mybir.dt.float8e4
    SEG = A0 + Y_NUM

    nc = bacc.Bacc("TRN2", target_bir_lowering=False, debug=False,
                   num_devices=N_CORES)
    # packed input layout (columns):
    #   [ lhsT_mc1 0:128 | W0 128:640 | W1 640:1152 |
    #     lhsT_mc0 1152:1280 | W2 1280:1792 | W3 1792:2304 ]
    # so each input DMA is 1152 cols and the first one carries everything
    # the first four matmuls (row block mc1, W blocks 0-1) need.
    aw = nc.declare_dram_parameter("aw", [KP, SEG], bf16, isOutput=False)
    LHS = {1: (0, 128), 0: (1152, 1280)}
    WOF = (128, 640, 1280, 1792)
    odts = (fp8, bf16) if hybrid else (bf16, bf16)
    outs = [nc.declare_dram_parameter(f"out{mc}", [128, Y_NUM], odts[mc],
                                      isOutput=True)
            for mc in range(NM)]

    # (mc, matmul piece widths, out tile widths): the bf16 block (mc1,
    # lhsT in the first input DMA) goes first; its leading 512 columns
    # are split into two 256-wide matmuls — the narrow head ramps the PE
    # p-state so every subsequent matmul runs at full speed, and the
    # first copies start ~400ns earlier. The fp8 block is split 1536+512
    # so the last transfer is small and gated only by the final copy.
    if hybrid:
        plan = ((1, (256, 256, 512, 512, 512), (1024, 1024)),
                (0, (512,) * 4, (1536, 512)))
        copy_eng = ('v', 's', 'v', 's', 'v', 's', 's', 'v', 's')
    else:
        plan = ((1, (512,) * 4, (1024, 1024)),
                (0, (512,) * 4, (1024, 1024)))
        copy_eng = ('v', 's') * 4

    with tile.TileContext(nc) as tc:
        with (
            tc.tile_pool(name="wt", bufs=1) as wpool,
            tc.tile_pool(name="io", bufs=4) as iopool,
            tc.tile_pool(name="ps", bufs=6, space="PSUM") as pspool,
        ):
            big = wpool.tile([KP, SEG], bf16, tag="big", name="big")

            nc.sync.dma_start(big[:, 0:1152], aw[:, 0:1152])
            nc.sync.dma_start(big[:, 1152:SEG], aw[:, 1152:SEG])

            # NOTE: only DVE and Act can read PSUM — a gpsimd copy passes
            # TimelineSim but fails neuronxcc lowering.
            COPY = {'v': lambda d, s_: nc.vector.tensor_copy(d, s_),
                    's': lambda d, s_: nc.scalar.copy(d, s_)}

            ci = 0
            for mc, pieces, widths in plan:
                odt = odts[mc]
                l0, l1 = LHS[mc]
                ti, tcol, tfill, ot = 0, 0, 0, None
                pcol = 0
                for pi, pw in enumerate(pieces):
                    w0 = WOF[pcol // 512] + pcol % 512
                    ps = pspool.tile([128, pw], f32, tag="ps",
                                     name=f"ps_{mc}_{pi}")
                    nc.tensor.matmul(
                        ps[:],
                        big[:, l0:l1],
                        big[:, w0:w0 + pw],
                        start=True, stop=True)
                    if ot is None:
                        ot = iopool.tile([128, widths[ti]], odt, tag="ot",
                                         name=f"ot_{mc}_{ti}")
                    COPY[copy_eng[ci]](ot[:, tfill:tfill + pw], ps[:])
                    ci += 1
                    tfill += pw
                    pcol += pw
                    if tfill == widths[ti]:
                        nc.sync.dma_start(
                            outs[mc][:, tcol:tcol + widths[ti]], ot[:])
                        tcol += widths[ti]
                        ti += 1
                        tfill, ot = 0, None

    nc.compile()
    return nc


def _get_compiled(KP, hybrid):
    key = (KP, hybrid)
    if key not in _COMPILED:
        _COMPILED[key] = _build_bass(KP, hybrid)
    return _COMPILED[key]


def _host_prep(t, y0, weights, ratios):
    """f64 host math: tap matrix C (SAMPLE_NUM x TAPS) and forcing s."""
    a = float(weights[0]) * float(ratios[0])
    b = float(weights[1]) * float(ratios[1])
    c = float(weights[2]) * float(ratios[2])

    t = t.astype(np.float32)
    steps_f32 = np.diff(t)                       # f32, as the reference
    sub_f32 = steps_f32 / np.float32(STEP_N)     # f32: big_step / step_n
    sub = sub_f32.astype(np.float64)
    alpha = 1.0 - sub * b
    beta = sub * a
    lam = alpha + beta

    # forcing: g_n accumulated over the 8 sub-steps with f32 time accrual
    # (tc advances in f32 exactly like the reference's scan carry)
    n = SAMPLE_NUM - 1
    gacc = np.zeros(n, dtype=np.float64)
    tc = t[:-1].copy()
    for _ in range(STEP_N):
        gacc = gacc * lam + sub * c * np.sin(tc.astype(np.float64))
        tc = tc + sub_f32
    s = np.zeros(SAMPLE_NUM, dtype=np.float64)
    lam8 = lam ** STEP_N
    for i in range(n):
        s[i + 1] = lam8[i] * s[i] + gacc[i]

    # taps: per big step the operator is sum_j C(8,j) alpha^(8-j) beta^j P^j
    binw = np.array([math.comb(STEP_N, j) for j in range(STEP_N + 1)])
    JMAX = 512
    C = np.zeros((SAMPLE_NUM, JMAX), dtype=np.float64)
    cur = np.zeros(JMAX, dtype=np.float64)
    cur[0] = 1.0
    C[0] = cur
    apow = alpha[:, None] ** np.arange(STEP_N, -1, -1.0)[None, :]
    bpow = beta[:, None] ** np.arange(0.0, STEP_N + 1.0)[None, :]
    wall = binw[None, :] * apow * bpow  # (n, 9)
    new = np.empty(JMAX, dtype=np.float64)
    for i in range(n):
        w = wall[i]
        new[:] = w[0] * cur
        for j in range(1, STEP_N + 1):
            new[j:] += w[j] * cur[:JMAX - j]
        cur, new = new, cur
        C[i + 1] = cur

    # band width: smallest TAPS in {127, 255, 511} such that the dropped
    # tail is negligible
    mass = np.maximum(np.abs(C).sum(axis=1), 1e-300)
    for TAPS in (127, 255, 511):
        tail = np.abs(C[:, TAPS - 8:TAPS + 1]).sum(axis=1) / mass
        if TAPS == JMAX - 1 or tail.max() < 1e-12:
            break

    return C[:, :TAPS].copy(), s


def kernel(t, y0, weights, ratios):
    import ml_dtypes

    t = np.asarray(t, dtype=np.float32)
    y0 = np.asarray(y0, dtype=np.float32)
    weights = np.asarray(weights, dtype=np.float32)
    ratios = np.asarray(ratios, dtype=np.float32)
    assert t.shape == (SAMPLE_NUM,) and y0.shape == (Y_NUM,)

    C, s = _host_prep(t, y0, weights, ratios)   # C: (2048, TAPS) f64
    TAPS = C.shape[1]

    # low-rank factorization of the row-normalized tap matrix
    rn = np.maximum(np.abs(C).sum(axis=1), 1e-300)
    U, S, Vt = np.linalg.svd(C / rn[:, None], full_matrices=False)
    S = np.maximum(S, 0.0)
    thr = S[0] * 1e-11
    R = max(int((S > thr).sum()), 1)
    KP = 32
    while KP - 1 < R and KP < 128:
        KP *= 2
    R = min(R, KP - 1)

    # fp8 is safe for each core's first 128 rows iff those rows carry a
    # tiny fraction of the output power (strong geometric row growth).
    pw = rn ** 2
    m8 = np.zeros(SAMPLE_NUM, bool)
    for q in range(N_CORES):
        m8[q * ROWS_PER_CORE:q * ROWS_PER_CORE + 128] = True
    frac = float(pw[m8].sum() / max(pw.sum(), 1e-300))
    hybrid = 0.027 * math.sqrt(frac) < 6e-3

    # A/rn = U S exactly; the host scales rows back by rn afterwards.
    As = U[:, :R] * S[:R]                        # (2048, R) f64, O(1) rows
    # W = V' G contracted on host: W[r, i] = sum_k Vt[r, k] y0[(i-k)%N]
    idx = (np.arange(Y_NUM)[None, :] - np.arange(TAPS)[:, None]) % Y_NUM
    G = y0[idx].astype(np.float64)              # (TAPS, 2048)
    W = Vt[:R] @ G                              # (R, 2048) f64

    # augment bias (A col R = s/rn, W row R = ones), zero-pad to KP
    Aa = np.zeros((SAMPLE_NUM, KP), dtype=np.float32)
    Aa[:, :R] = As
    Aa[:, R] = s / rn
    Wa = np.zeros((KP, Y_NUM), dtype=np.float32)
    Wa[:R] = W
    Wa[R] = 1.0

    Wb = Wa.astype(ml_dtypes.bfloat16)           # (KP, 2048)

    nc = _get_compiled(KP, hybrid)
    core_ids = list(range(N_CORES))
    in_maps = []
    for q in core_ids:
        rows = slice(q * ROWS_PER_CORE, (q + 1) * ROWS_PER_CORE)
        Ab = np.ascontiguousarray(Aa[rows].T).astype(ml_dtypes.bfloat16)
        # interleaved layout matching the builder (see _build_bass)
        in_maps.append({"aw": np.ascontiguousarray(np.concatenate(
            [Ab[:, 128:256], Wb[:, 0:1024],
             Ab[:, 0:128], Wb[:, 1024:2048]], axis=1))})

    from concourse.bass_utils import run_bass_kernel_spmd
    res = run_bass_kernel_spmd(nc, in_maps, core_ids)
    out = np.empty((SAMPLE_NUM, Y_NUM), dtype=np.float32)
    for q in core_ids:
        r0 = q * ROWS_PER_CORE
        out[r0:r0 + 128] = res.results[q]["out0"].astype(np.float32)
        out[r0 + 128:r0 + 256] = res.results[q]["out1"].astype(np.float32)
    out *= rn[:, None].astype(np.float32)
    return out


# revision 18
# speedup vs baseline: 1.0101x; 1.0101x over previous
"""NeuroODE kernel for 8 Trainium2 NeuronCores.

Math: each Euler sub-step is y <- (alpha*I + beta*P) y + gamma*ones, with
P the cyclic shift (roll by 1). Composing the 8 sub-steps of big step n
gives a 9-tap circulant operator W_n; composing across big steps keeps the
state circulant in y0:

    y_n = C_n (*) y0 + s_n * ones

where C_n (tap vector, circular convolution) obeys C_{n+1} = W_n (*) C_n
and the forcing collapses to the scalar recurrence s_{n+1} = lam_n^8 s_n
+ g_n because P*ones = ones (computed on host in f64). The taps are a
binomial bump centered at ~8*n*beta/(alpha+beta), so C_n is supported on
the first TAPS taps, and the full output is the banded product

    Y[n, i] = sum_k C[n, k] * y0[(i - k) mod 2048] + s_n.

The row-normalized tap matrix is a smooth one-parameter family of
binomial bumps with numerical rank ~25, so C = D @ (U S V'); the device
never sees C or the shifted-y0 matrix at all:

    Y = A @ W + s 1',   A = D U S (2048 x R),  W = V' G (R x 2048)

with G[k, i] = y0[(i-k) mod 2048] contracted on the host (tiny, f64).
The bias is folded in as an extra contraction row. Each of the 8 cores
computes 256 output rows (its row block of A against the shared W).

Precision: operands are plain bf16 and A is shipped row-normalized
(A/rn = U S exactly), so the device computes Y/rn and the host scales
rows back by rn after the gather. The shipped output is bf16 for each
core's last 128 rows and fp8-e4m3 for its first 128: the row norms grow
geometrically in n (lam > 1), so the first half of every 256-row block
carries ~3% of that block's output power and tolerates fp8's ~2.7% rms
element error at negligible norm-level cost (measured 5.5e-3 total vs
the f32 reference, against a 2e-2 gate, while cutting the dominant
output-DMA bytes by 25%). If a different parameter draw made the row
norms flat or decaying, the host detects it and falls back to an
all-bf16 variant (measured 2.8e-3).

Schedule (TimelineSim-tuned): operands packed per core as
[lhsT_mc1 | W0 | W1 | lhsT_mc0 | W2 | W3] in one [KP, 2304] bf16
tensor, loaded in two equal DMAs: the first carries everything the
bf16 row block's leading matmuls (lhsT_mc1 x W0-W1) need, so no matmul
stalls on the second DMA's 900ns completion semaphore. The bf16 row block is
computed first (its 2x728ns out-DMAs start the transfer chain early),
the fp8 block second as a 1536+512 split so the last transfer is small
and gated only by the final copy. PSUM->SBUF cast copies alternate
DVE/Act (the only PSUM-capable engines); all out-DMAs go on the SP
queue - the 650ns dispatch + 625ns shared-HWDGE staircase stays just
ahead of the transfers, keeping the DMA engines dense from first
transfer to end.
"""

import math

import numpy as np

SAMPLE_NUM = 2048
Y_NUM = 2048
STEP_N = 8
N_CORES = 8
ROWS_PER_CORE = SAMPLE_NUM // N_CORES  # 256
A0 = ROWS_PER_CORE                     # lhsT columns in the packed input
NM = ROWS_PER_CORE // 128              # 128-row output row blocks

_COMPILED = {}  # (KP, hybrid) -> nc


def _build_bass(KP, hybrid):
    """KP: padded contraction size (rank+bias+pad): 32/64/128."""
    import concourse.tile as tile
    from concourse import bacc, mybir

    f32 = mybir.dt.float32
    bf16 = mybir.dt.bfloat16
    fp8 = mybir.dt.float8e4
    SEG = A0 + Y_NUM

    nc = bacc.Bacc("TRN2", target_bir_lowering=False, debug=False,
                   num_devices=N_CORES)
    # packed input layout (columns):
    #   [ lhsT_mc1 0:128 | W0 128:640 | W1 640:1152 |
    #     lhsT_mc0 1152:1280 | W2 1280:1792 | W3 1792:2304 ]
    # so each input DMA is 1152 cols and the first one carries everything
    # the first four matmuls (row block mc1, W blocks 0-1) need.
    aw = nc.declare_dram_parameter("aw", [KP, SEG], bf16, isOutput=False)
    LHS = {1: (0, 128), 0: (1152, 1280)}
    WOF = (128, 640, 1280, 1792)
    odts = (fp8, bf16) if hybrid else (bf16, bf16)
    outs = [nc.declare_dram_parameter(f"out{mc}", [128, Y_NUM], odts[mc],
                                      isOutput=True)
            for mc in range(NM)]

    # (mc, matmul piece widths, out tile widths): the bf16 block (mc1,
    # lhsT in the first input DMA) goes first; its leading 512 columns
    # are split into two 256-wide matmuls — the narrow head ramps the PE
    # p-state so every subsequent matmul runs at full speed, and the
    # first copies start ~400ns earlier. The fp8 block is split 1536+512
    # so the last transfer is small and gated only by the final copy.
    if hybrid:
        plan = ((1, (256, 256, 512, 512, 512), (1024, 1024)),
                (0, (512,) * 4, (1536, 512)))
        copy_eng = ('v', 's', 'v', 's', 'v', 's', 's', 'v', 's')
    else:
        plan = ((1, (512,) * 4, (1024, 1024)),
                (0, (512,) * 4, (1024, 1024)))
        copy_eng = ('v', 's') * 4

    with tile.TileContext(nc) as tc:
        with (
            tc.tile_pool(name="wt", bufs=1) as wpool,
            tc.tile_pool(name="io", bufs=4) as iopool,
            tc.tile_pool(name="ps", bufs=6, space="PSUM") as pspool,
        ):
            big = wpool.tile([KP, SEG], bf16, tag="big", name="big")

            nc.sync.dma_start(big[:, 0:1152], aw[:, 0:1152])
            nc.sync.dma_start(big[:, 1152:SEG], aw[:, 1152:SEG])

            # NOTE: only DVE and Act can read PSUM — a gpsimd copy passes
            # TimelineSim but fails neuronxcc lowering.
            COPY = {'v': lambda d, s_: nc.vector.tensor_copy(d, s_),
                    's': lambda d, s_: nc.scalar.copy(d, s_)}

            ci = 0
            for mc, pieces, widths in plan:
                odt = odts[mc]
                l0, l1 = LHS[mc]
                ti, tcol, tfill, ot = 0, 0, 0, None
                pcol = 0
                for pi, pw in enumerate(pieces):
                    w0 = WOF[pcol // 512] + pcol % 512
                    ps = pspool.tile([128, pw], f32, tag="ps",
                                     name=f"ps_{mc}_{pi}")
                    nc.tensor.matmul(
                        ps[:],
                        big[:, l0:l1],
                        big[:, w0:w0 + pw],
                        start=True, stop=True)
                    if ot is None:
                        ot = iopool.tile([128, widths[ti]], odt, tag="ot",
                                         name=f"ot_{mc}_{ti}")
                    COPY[copy_eng[ci]](ot[:, tfill:tfill + pw], ps[:])
                    ci += 1
                    tfill += pw
                    pcol += pw
                    if tfill == widths[ti]:
                        nc.sync.dma_start(
                            outs[mc][:, tcol:tcol + widths[ti]], ot[:])
                        tcol += widths[ti]
                        ti += 1
                        tfill, ot = 0, None

    nc.compile()
    return nc


def _get_compiled(KP, hybrid):
    key = (KP, hybrid)
    if key not in _COMPILED:
        _COMPILED[key] = _build_bass(KP, hybrid)
    return _COMPILED[key]


def _host_prep(t, y0, weights, ratios):
    """f64 host math: tap matrix C (SAMPLE_NUM x TAPS) and forcing s."""
    a = float(weights[0]) * float(ratios[0])
    b = float(weights[1]) * float(ratios[1])
    c = float(weights[2]) * float(ratios[2])

    t = t.astype(np.float32)
    steps_f32 = np.diff(t)                       # f32, as the reference
    sub_f32 = steps_f32 / np.float32(STEP_N)     # f32: big_step / step_n
    sub = sub_f32.astype(np.float64)
    alpha = 1.0 - sub * b
    beta = sub * a
    lam = alpha + beta

    # forcing: g_n accumulated over the 8 sub-steps with f32 time accrual
    # (tc advances in f32 exactly like the reference's scan carry)
    n = SAMPLE_NUM - 1
    gacc = np.zeros(n, dtype=np.float64)
    tc = t[:-1].copy()
    for _ in range(STEP_N):
        gacc = gacc * lam + sub * c * np.sin(tc.astype(np.float64))
        tc = tc + sub_f32
    s = np.zeros(SAMPLE_NUM, dtype=np.float64)
    lam8 = lam ** STEP_N
    for i in range(n):
        s[i + 1] = lam8[i] * s[i] + gacc[i]

    # taps: per big step the operator is sum_j C(8,j) alpha^(8-j) beta^j P^j
    binw = np.array([math.comb(STEP_N, j) for j in range(STEP_N + 1)])
    JMAX = 512
    C = np.zeros((SAMPLE_NUM, JMAX), dtype=np.float64)
    cur = np.zeros(JMAX, dtype=np.float64)
    cur[0] = 1.0
    C[0] = cur
    apow = alpha[:, None] ** np.arange(STEP_N, -1, -1.0)[None, :]
    bpow = beta[:, None] ** np.arange(0.0, STEP_N + 1.0)[None, :]
    wall = binw[None, :] * apow * bpow  # (n, 9)
    new = np.empty(JMAX, dtype=np.float64)
    for i in range(n):
        w = wall[i]
        new[:] = w[0] * cur
        for j in range(1, STEP_N + 1):
            new[j:] += w[j] * cur[:JMAX - j]
        cur, new = new, cur
        C[i + 1] = cur

    # band width: smallest TAPS in {127, 255, 511} such that the dropped
    # tail is negligible
    mass = np.maximum(np.abs(C).sum(axis=1), 1e-300)
    for TAPS in (127, 255, 511):
        tail = np.abs(C[:, TAPS - 8:TAPS + 1]).sum(axis=1) / mass
        if TAPS == JMAX - 1 or tail.max() < 1e-12:
            break

    return C[:, :TAPS].copy(), s


def kernel(t, y0, weights, ratios):
    import ml_dtypes

    t = np.asarray(t, dtype=np.float32)
    y0 = np.asarray(y0, dtype=np.float32)
    weights = np.asarray(weights, dtype=np.float32)
    ratios = np.asarray(ratios, dtype=np.float32)
    assert t.shape == (SAMPLE_NUM,) and y0.shape == (Y_NUM,)

    C, s = _host_prep(t, y0, weights, ratios)   # C: (2048, TAPS) f64
    TAPS = C.shape[1]

    # low-rank factorization of the row-normalized tap matrix
    rn = np.maximum(np.abs(C).sum(axis=1), 1e-300)
    U, S, Vt = np.linalg.svd(C / rn[:, None], full_matrices=False)
    S = np.maximum(S, 0.0)
    # truncate to the numerically needed rank: singular values below
    # ~3e-4 of S[0] are buried under the bf16/fp8 quantization noise
    # (measured: rank-16 gives 5.509e-3 total vs rank-31's 5.500e-3),
    # and a smaller contraction dim shrinks the input DMAs that anchor
    # the whole pipeline.
    R = max(int((S > S[0] * 3e-4).sum()), 8)
    R = min(R, 127)
    KP = R + 1

    # fp8 is safe for each core's first 128 rows iff those rows carry a
    # tiny fraction of the output power (strong geometric row growth).
    pw = rn ** 2
    m8 = np.zeros(SAMPLE_NUM, bool)
    for q in range(N_CORES):
        m8[q * ROWS_PER_CORE:q * ROWS_PER_CORE + 128] = True
    frac = float(pw[m8].sum() / max(pw.sum(), 1e-300))
    hybrid = 0.027 * math.sqrt(frac) < 6e-3

    # A/rn = U S exactly; the host scales rows back by rn afterwards.
    As = U[:, :R] * S[:R]                        # (2048, R) f64, O(1) rows
    # W = V' G contracted on host: W[r, i] = sum_k Vt[r, k] y0[(i-k)%N]
    idx = (np.arange(Y_NUM)[None, :] - np.arange(TAPS)[:, None]) % Y_NUM
    G = y0[idx].astype(np.float64)              # (TAPS, 2048)
    W = Vt[:R] @ G                              # (R, 2048) f64

    # augment bias (A col R = s/rn, W row R = ones), zero-pad to KP
    Aa = np.zeros((SAMPLE_NUM, KP), dtype=np.float32)
    Aa[:, :R] = As
    Aa[:, R] = s / rn
    Wa = np.zeros((KP, Y_NUM), dtype=np.float32)
    Wa[:R] = W
    Wa[R] = 1.0

    Wb = Wa.astype(ml_dtypes.bfloat16)           # (KP, 2048)

    nc = _get_compiled(KP, hybrid)
    core_ids = list(range(N_CORES))
    in_maps = []
    for q in core_ids:
        rows = slice(q * ROWS_PER_CORE, (q + 1) * ROWS_PER_CORE)
        Ab = np.ascontiguousarray(Aa[rows].T).astype(ml_dtypes.bfloat16)
        # interleaved layout matching the builder (see _build_bass)
        in_maps.append({"aw": np.ascontiguousarray(np.concatenate(
            [Ab[:, 128:256], Wb[:, 0:1024],
             Ab[:, 0:128], Wb[:, 1024:2048]], axis=1))})

    from concourse.bass_utils import run_bass_kernel_spmd
    res = run_bass_kernel_spmd(nc, in_maps, core_ids)
    out = np.empty((SAMPLE_NUM, Y_NUM), dtype=np.float32)
    for q in core_ids:
        r0 = q * ROWS_PER_CORE
        out[r0:r0 + 128] = res.results[q]["out0"].astype(np.float32)
        out[r0 + 128:r0 + 256] = res.results[q]["out1"].astype(np.float32)
    out *= rn[:, None].astype(np.float32)
    return out
